# revision 13
# baseline (speedup 1.0000x reference)
"""Trainium2 Bass kernel for 16-head causal MHA (B=2, T=2048, D=1024, fp32 I/O).

Sharding: batch x head-group tensor parallel. Core c owns batch c//4 and the
4 heads of group c%4: it gets that batch's x^T, Wq/Wk/Wv column slices
[:, 256g:256g+256] and the Wo row slice [256g:256g+256, :], computes its 4
heads' attention, and produces a partial output [2048, 1024]; the host sums
4 partials per batch in f64.

Per-core device program, matmul inputs bf16 (1 PE cycle/col), fp32 PSUM:
  - Q^T/K^T = W.T @ x^T per 128-row half (heads 2h,2h+1 stacked dk=64 each),
    weights stationary, N=512 moving blocks.
  - V natural = x^T-chunk-stationary @ Wv, N=256 blocks -> v_sb[128,kc,256].
  - attention in S^T layout over (qn, kc) blocks of [128 keys, 512 queries]:
    the head-pair shares one [128,1024] 2-bank PSUM tile (row groups 0-63 /
    64-127 run concurrently on the PE); one exp ACTIVATE per pair covers
    both heads; causality = skipping dead blocks + a 0/1 staircase mask
    multiply on diagonal blocks (VectorE).
  - ctx^T: the head-pair's two M=64 accumulations go to output partitions
    0-63 / 64-127 of one PSUM bank (column-group concurrency); softmax
    denominators ride as four concurrent M=1 matmuls (ones-column lhsT)
    into col positions {0,32,64,96} of a second bank.
  - 1/den = exp(-ln den) on ScalarE over the 4 head-rows at once; the
    broadcast across 64 partitions per head is ONE K=4 matmul against a
    block-ones "expand" matrix; normalization in-place on VectorE.
  - out partial = ctxt^T.T @ Wo, K=256 as two accumulating matmuls.

Scheduling: emission order interleaves the later Q/K rc-blocks and V chunks
into the ScalarE-bound attention loop as PE filler, and runs each qn's
normalize + output projection + DMA inside the next qn's attention, so the
PE never idles long enough for the HAM clock gate to re-throttle.

Infrastructure: the external walrus allows only ONE sync wait per
instruction; Tile emits more, so a post-pass hoists extra waits onto
single-wait no-ops and the TileContext closing drain is split into a chain
of single-wait drains.
"""

import numpy as np

import bass_rust
from bass_rust import ScopedClock
import concourse.bass as bass
import concourse.mybir as mybir
import concourse.tile as tile

F32 = mybir.dt.float32
BF16 = mybir.dt.bfloat16
F32R = mybir.dt.float32r
B, T, D = 2, 2048, 1024
NCORES = 8
P = 128          # partitions
FC = D // P      # 8 feature chunks
QW = 512         # query block width
QN = T // QW     # 4 query blocks
KC = T // P      # 16 key chunks
NH = 4           # heads per core
DK = 64
HW = NH * DK     # 256: per-core qkv width

# ---------------------------------------------------------------------------
# TileContext drain fix: the external walrus in this container allows only ONE
# sync wait per instruction, but Tile's closing drain packs one wait per active
# proc. Split it into a chain of single-wait drains (same semantics).
_PATCHED = False


def _patched_drain_and_barrier(self, tick_clock, wait_clock):
    nc = self.nc
    drain_inst = nc.sync.drain()
    wait_clock.add_sem_waits(
        drain_inst.ins, ScopedClock({None: tick_clock.global_clock})
    )
    si = drain_inst.ins.sync_info
    waits = list(si.on_wait) if si is not None else []
    if len(waits) > 1:
        si.on_wait = [waits[0]]
        drain_inst.ins.sync_info = si
        for w in waits[1:]:
            d2 = nc.sync.drain()
            si2 = d2.ins.sync_info
            if si2 is None:
                si2 = bass_rust.SyncInfo(on_wait=[w], on_update=[])
            else:
                si2.on_wait = [w]
            d2.ins.sync_info = si2
    nc.all_engine_barrier()
    assert self.sems is not None
    popped = nc._tile_sem_poison_stack.pop()
    assert popped is self._sem_poison
    nc.clear_and_free_semaphores(list(self.sems.allocated().values()))
    nc.all_engine_barrier()


def _apply_tile_patch():
    global _PATCHED
    if not _PATCHED:
        tile.TileContext._drain_and_barrier = _patched_drain_and_barrier
        _PATCHED = True


def _split_multi_waits(nc):
    """Post-pass: the external walrus accepts only 1 sync wait per
    instruction (2 for EventSemaphore). Tile emits more. Hoist extra waits
    onto same-engine no-ops inserted just before."""
    for f in nc.m.functions:
        for bb in f.blocks:
            new = []
            for ins in bb.instructions:
                si = ins.sync_info
                if si is not None:
                    cap = 2 if isinstance(ins, mybir.InstEventSemaphore) else 1
                    waits = list(si.on_wait)
                    if len(waits) > cap:
                        for w in waits[:-cap]:
                            nop = mybir.InstNoOp(
                                name=nc.get_next_instruction_name(),
                                engine=ins.engine,
                                sync_info=bass_rust.SyncInfo(
                                    on_wait=[w], on_update=[]
                                ),
                                bass_nofuse=True,
                            )
                            nc.register_instruction(nop, overwrite=True)
                            new.append(nop)
                        si.on_wait = waits[-cap:]
                        ins.sync_info = si
                new.append(ins)
            bb.instructions = new


# ---------------------------------------------------------------------------
_PROGRAM = None


def build_program():
    global _PROGRAM
    if _PROGRAM is not None:
        return _PROGRAM
    _apply_tile_patch()
    Exp = mybir.ActivationFunctionType.Exp
    Log = mybir.ActivationFunctionType.Ln

    nc = bass.Bass()
    xt_d = nc.declare_dram_parameter("xt", [D, T], BF16, isOutput=False)
    wq_d = nc.declare_dram_parameter("wq", [D, HW], BF16, isOutput=False)
    wk_d = nc.declare_dram_parameter("wk", [D, HW], BF16, isOutput=False)
    wv_d = nc.declare_dram_parameter("wv", [D, HW], BF16, isOutput=False)
    wo_d = nc.declare_dram_parameter("wo", [HW, D], BF16, isOutput=False)
    mask_d = nc.declare_dram_parameter("mask", [P, 4, 1024], BF16, isOutput=False)
    e4_d = nc.declare_dram_parameter("e4", [P, HW], F32R, isOutput=False)
    out_d = nc.declare_dram_parameter("out", [T, D], F32, isOutput=True)

    with tile.TileContext(nc) as tc:
        from contextlib import ExitStack

        ctx = ExitStack()
        with ctx:
            consts = ctx.enter_context(tc.tile_pool(name="consts", bufs=1))
            xt_pool = ctx.enter_context(tc.tile_pool(name="xt", bufs=8))
            qk_pool = ctx.enter_context(tc.tile_pool(name="qk", bufs=1))
            v_pool = ctx.enter_context(tc.tile_pool(name="v", bufs=1))
            exp_pool = ctx.enter_context(tc.tile_pool(name="exp", bufs=6))
            ctxt_pool = ctx.enter_context(tc.tile_pool(name="ctxt", bufs=1))
            den_pool = ctx.enter_context(tc.tile_pool(name="den", bufs=1))
            ob_pool = ctx.enter_context(tc.tile_pool(name="ob", bufs=3))

            ps_big = ctx.enter_context(
                tc.tile_pool(name="ps_big", bufs=2, space="PSUM")
            )
            ps_ctx = ctx.enter_context(
                tc.tile_pool(name="ps_ctx", bufs=2, space="PSUM")
            )
            ps_den = ctx.enter_context(
                tc.tile_pool(name="ps_den", bufs=2, space="PSUM")
            )

            # ---- constants ----
            wq_sb = consts.tile([P, FC, HW], BF16, tag="wq")
            wk_sb = consts.tile([P, FC, HW], BF16, tag="wk")
            wv_sb = consts.tile([P, FC, HW], BF16, tag="wv")
            wo_sb = consts.tile([P, 2, D], BF16, tag="wo")
            mask_sb = consts.tile([P, 4, 1024], BF16, tag="mask")
            e4_sb = consts.tile([P, HW], F32R, tag="e4")
            onescol = consts.tile([P, 1], BF16, tag="onescol")
            # xt chunks + wq/wk trigger first on the sync queue (they gate
            # the first projections); bulky later-phase consts go through the
            # gpsimd queue so their descriptor-gen does not delay xt.
            xts = []
            for fc in range(2):
                xt_t = xt_pool.tile([P, T], BF16, tag="xt", name=f"xt{fc}")
                nc.sync.dma_start(out=xt_t, in_=xt_d[fc * P : (fc + 1) * P, :])
                xts.append(xt_t)
            nc.sync.dma_start(out=wq_sb, in_=wq_d.rearrange("(f p) c -> p f c", p=P))
            nc.sync.dma_start(out=wk_sb, in_=wk_d.rearrange("(f p) c -> p f c", p=P))
            for fc in range(2, FC):
                xt_t = xt_pool.tile([P, T], BF16, tag="xt", name=f"xt{fc}")
                nc.sync.dma_start(out=xt_t, in_=xt_d[fc * P : (fc + 1) * P, :])
                xts.append(xt_t)
            nc.gpsimd.dma_start(out=wv_sb, in_=wv_d.rearrange("(f p) c -> p f c", p=P))
            nc.gpsimd.dma_start(out=wo_sb, in_=wo_d.rearrange("(k p) c -> p k c", p=P))
            nc.gpsimd.dma_start(out=mask_sb, in_=mask_d[:, :, :])
            nc.gpsimd.dma_start(out=e4_sb, in_=e4_d[:, :])
            nc.vector.memset(onescol, 1.0)

            # persistent SBUF tensors
            qt = [qk_pool.tile([P, T], BF16, tag=f"qt{h}", name=f"qt{h}") for h in range(2)]
            kt = [qk_pool.tile([P, T], BF16, tag=f"kt{h}", name=f"kt{h}") for h in range(2)]
            v_sb = v_pool.tile([P, KC, HW], BF16, tag="v")
            ctxt = [ctxt_pool.tile([P, T], BF16, tag=f"ctxt{k}", name=f"ctxt{k}") for k in range(2)]
            lnden = den_pool.tile([P, T], F32, tag="lnden")
            rcp = den_pool.tile([P, T], F32R, tag="rcp")

            # ---- PE work units (emitted directly or as attention filler) ----
            def qk_unit(dst, w_sb, hp, rc):
                """One [128,512] Q^T or K^T block: 8 accumulating matmuls."""
                ps = ps_big.tile([P, 2 * QW], F32, tag="big", name=f"qk{id(dst)}{hp}{rc}")
                for fc in range(FC):
                    nc.tensor.matmul(
                        ps[:, 0:QW],
                        lhsT=w_sb[:, fc, hp * P : (hp + 1) * P],
                        rhs=xts[fc][:, rc * QW : (rc + 1) * QW],
                        start=(fc == 0),
                        stop=(fc == FC - 1),
                    )
                nc.vector.tensor_copy(dst[:, rc * QW : (rc + 1) * QW], ps[:, 0:QW])

            def v_unit(kc):
                """Two key chunks of V [128 tok, 256 dv each]: 16 matmuls."""
                ps = ps_big.tile([P, 2 * QW], F32, tag="big", name=f"v{kc}")
                for i in range(2):
                    for fc in range(FC):
                        nc.tensor.matmul(
                            ps[:, i * HW : (i + 1) * HW],
                            lhsT=xts[fc][:, (kc + i) * P : (kc + i + 1) * P],
                            rhs=wv_sb[:, fc, :],
                            start=(fc == 0),
                            stop=(fc == FC - 1),
                        )
                nc.vector.tensor_copy(
                    v_sb[:, kc : kc + 2, :],
                    ps[:, 0 : 2 * HW].rearrange("p (i c) -> p i c", i=2),
                )

            # ---- upfront projections: only the rc0 Q/K blocks ----
            for hp in range(2):
                qk_unit(qt[hp], wq_sb, hp, 0)
                qk_unit(kt[hp], wk_sb, hp, 0)

            # filler schedule: work units injected into qn's attention iters
            # (after each iteration's S matmuls, so they sit off the S->exp
            # critical path and fill the PE while ScalarE works)
            def qk_f(dst, w_sb, hp, rc):
                return lambda: qk_unit(dst, w_sb, hp, rc)

            fillers = {
                0: [lambda: v_unit(0), lambda: v_unit(2),
                    qk_f(qt[0], wq_sb, 0, 1), qk_f(kt[0], wk_sb, 0, 1),
                    qk_f(qt[1], wq_sb, 1, 1), qk_f(kt[1], wk_sb, 1, 1),
                    lambda: v_unit(4)],
                1: [qk_f(qt[0], wq_sb, 0, 2), qk_f(kt[0], wk_sb, 0, 2),
                    qk_f(qt[1], wq_sb, 1, 2), qk_f(kt[1], wk_sb, 1, 2),
                    lambda: v_unit(6), lambda: v_unit(8)],
                2: [qk_f(qt[0], wq_sb, 0, 3), qk_f(kt[0], wk_sb, 0, 3),
                    qk_f(qt[1], wq_sb, 1, 3), qk_f(kt[1], wk_sb, 1, 3),
                    lambda: v_unit(10), lambda: v_unit(12)],
                3: [lambda: v_unit(14)],
            }

            # deferred epilogue emitters, run interleaved into the next qn
            pending_epilogue = []

            def emit_epilogue(qn, ctx_ts, den_ps):
                steps = []
                qs = slice(qn * QW, (qn + 1) * QW)

                def s_evac():
                    for hp in range(2):
                        nc.vector.tensor_copy(ctxt[hp][:, qs], ctx_ts[hp])

                def s_rcp():
                    nc.scalar.activation(
                        out=lnden[:, qs], in_=den_ps, func=Log
                    )
                    nc.scalar.activation(
                        out=rcp[:, qs], in_=lnden[:, qs], func=Exp, scale=-1.0
                    )

                def s_norm():
                    bc = ps_big.tile([P, 2 * QW], F32, tag="big", name=f"bc{qn}")
                    for hp in range(2):
                        nc.tensor.matmul(
                            bc[:, hp * QW : (hp + 1) * QW],
                            lhsT=e4_sb[:, hp * P : (hp + 1) * P],
                            rhs=rcp[:, qs],
                            start=True,
                            stop=True,
                        )
                    for hp in range(2):
                        nc.vector.tensor_mul(
                            ctxt[hp][:, qs],
                            ctxt[hp][:, qs],
                            bc[:, hp * QW : (hp + 1) * QW],
                        )

                def s_oproj(rc):
                    def run():
                        ps = ps_big.tile(
                            [P, 2 * QW], F32, tag="big", name=f"op{qn}{rc}"
                        )
                        tok = qn * QW + rc * P
                        for kch in range(2):
                            for c2 in range(2):
                                nc.tensor.matmul(
                                    ps[:, c2 * QW : (c2 + 1) * QW],
                                    lhsT=ctxt[kch][:, tok : tok + P],
                                    rhs=wo_sb[:, kch, c2 * QW : (c2 + 1) * QW],
                                    start=(kch == 0),
                                    stop=(kch == 1),
                                )
                        ob = ob_pool.tile([P, D], F32, tag="ob")
                        nc.vector.tensor_copy(ob, ps)
                        nc.sync.dma_start(out=out_d[tok : tok + P, :], in_=ob)
                    return run

                steps.append(s_evac)
                steps.append(s_rcp)
                steps.append(s_norm)
                for rc in range(4):
                    steps.append(s_oproj(rc))
                return steps

            # ---- attention ----
            for qn in range(QN):
                nkc = 4 * (qn + 1)
                qs = slice(qn * QW, (qn + 1) * QW)
                ctx_ts = [
                    ps_ctx.tile([P, QW], F32, tag="ctx", name=f"ctx{qn}{hp}")
                    for hp in range(2)
                ]
                den_ps = ps_den.tile([P, QW], F32, tag="den", name=f"den{qn}")
                nc.vector.memset(den_ps, 1.0)
                fill = list(fillers[qn]) + pending_epilogue
                pending_epilogue = []
                # spread filler work across this qn's kc iterations
                nfill = len(fill)
                for kc in range(nkc):
                    j = kc - 4 * qn
                    es = []
                    for hp in range(2):
                        s_ps = ps_big.tile(
                            [P, 2 * QW], F32, tag="big", name=f"s{qn}{kc}{hp}"
                        )
                        for h2 in range(2):
                            nc.tensor.matmul(
                                s_ps[:, h2 * QW : (h2 + 1) * QW],
                                lhsT=kt[hp][
                                    h2 * DK : (h2 + 1) * DK, kc * P : (kc + 1) * P
                                ],
                                rhs=qt[hp][h2 * DK : (h2 + 1) * DK, qs],
                                start=True,
                                stop=True,
                            )
                        e = exp_pool.tile([P, 2 * QW], BF16, tag="exp")
                        nc.scalar.activation(out=e, in_=s_ps, func=Exp, scale=0.125)
                        if j >= 0:  # diagonal: causal staircase mask
                            nc.vector.tensor_mul(e, e, mask_sb[:, j, :])
                        es.append(e)
                    lo = nfill * kc // nkc
                    hi = nfill * (kc + 1) // nkc
                    for f in fill[lo:hi]:
                        f()
                    first = kc == 0
                    last = kc == nkc - 1
                    for hp in range(2):
                        for h2 in range(2):
                            # start=True only for the bank's first write; the
                            # pair shares one bank (h2=1 overwrites-where-clear)
                            nc.tensor.matmul(
                                ctx_ts[hp][h2 * DK : (h2 + 1) * DK, :],
                                lhsT=v_sb[:, kc, (2 * hp + h2) * DK : (2 * hp + h2 + 1) * DK],
                                rhs=es[hp][:, h2 * QW : (h2 + 1) * QW],
                                start=first,
                                stop=last,
                                skip_group_check=True,
                            )
                    for hp in range(2):
                        for h2 in range(2):
                            h = 2 * hp + h2
                            nc.tensor.matmul(
                                den_ps[32 * h : 32 * h + 1, :],
                                lhsT=onescol,
                                rhs=es[hp][:, h2 * QW : (h2 + 1) * QW],
                                start=first,
                                stop=last,
                                skip_group_check=True,
                                tile_position=(0, 32 * h),
                            )
                pending_epilogue = emit_epilogue(qn, ctx_ts, den_ps)
            for f in pending_epilogue:
                f()

    _split_multi_waits(nc)
    _PROGRAM = nc
    return nc


def _make_mask():
    # mask4[i, j, 512h + q] = 1.0 if q >= 128j + i else 0.0  (shape [128, 4, 1024])
    i = np.arange(P)[:, None, None]
    j = np.arange(4)[None, :, None]
    q = np.arange(1024)[None, None, :] % 512
    return (q >= 128 * j + i).astype(np.float32)


def make_in_maps(x, Wq, Wk, Wv, Wo):
    import ml_dtypes

    nd = ml_dtypes.bfloat16
    x = np.asarray(x, dtype=np.float32)
    xt = np.ascontiguousarray(x.reshape(B * T, D).T).astype(nd)  # [1024, 4096]
    mask = _make_mask().astype(nd)
    pp = np.arange(P)[:, None]
    cc = np.arange(HW)[None, :]
    e4 = (pp == 32 * (cc // DK)).astype(np.float32)  # [128, 256] (f32r)
    Wq, Wk, Wv, Wo = (np.asarray(w, dtype=np.float32) for w in (Wq, Wk, Wv, Wo))
    in_maps = []
    for c in range(NCORES):
        b, hg = c // 4, c % 4
        cols = slice(hg * HW, (hg + 1) * HW)
        in_maps.append(
            {
                "xt": np.ascontiguousarray(xt[:, b * T : (b + 1) * T]),
                "wq": np.ascontiguousarray(Wq[:, cols]).astype(nd),
                "wk": np.ascontiguousarray(Wk[:, cols]).astype(nd),
                "wv": np.ascontiguousarray(Wv[:, cols]).astype(nd),
                "wo": np.ascontiguousarray(Wo[cols, :]).astype(nd),
                "mask": mask,
                "e4": e4,
            }
        )
    return in_maps


def kernel(x, Wq, Wk, Wv, Wo):
    from concourse.bass_utils import run_bass_kernel_spmd

    nc = build_program()
    in_maps = make_in_maps(x, Wq, Wk, Wv, Wo)
    res = run_bass_kernel_spmd(nc, in_maps, core_ids=list(range(NCORES)))
    out = np.zeros((B, T, D), dtype=np.float64)
    for c in range(NCORES):
        out[c // 4] += res.results[c]["out"]
    return out.astype(np.float32)


if __name__ == "__main__":
    rng = np.random.default_rng(0)
    s = 1.0 / np.sqrt(D)
    ins = {
        "x": rng.standard_normal((B, T, D)).astype(np.float32),
        "Wq": (rng.standard_normal((D, D)) * s).astype(np.float32),
        "Wk": (rng.standard_normal((D, D)) * s).astype(np.float32),
        "Wv": (rng.standard_normal((D, D)) * s).astype(np.float32),
        "Wo": (rng.standard_normal((D, D)) * (1.0 / np.sqrt(D))).astype(np.float32),
    }
    out = kernel(**ins)
    print("out", out.shape, out.dtype, float(np.abs(out).max()))


# revision 15
# speedup vs baseline: 1.1542x; 1.1542x over previous
"""Trainium2 Bass kernel for 16-head causal MHA (B=2, T=2048, D=1024, fp32 I/O).

Sharding: batch x head-group tensor parallel. Core c owns batch c//4 and the
4 heads of group c%4: it gets that batch's x^T, Wq/Wk/Wv column slices
[:, 256g:256g+256] and the Wo row slice [256g:256g+256, :], computes its 4
heads' attention, and produces a partial output [2048, 1024]; the host sums
4 partials per batch in f64.

Per-core device program, matmul inputs bf16 (1 PE cycle/col), fp32 PSUM:
  - Q^T/K^T = W.T @ x^T per 128-row half (heads 2h,2h+1 stacked dk=64 each),
    weights stationary, N=512 moving blocks.
  - V natural = x^T-chunk-stationary @ Wv, N=256 blocks -> v_sb[128,kc,256].
  - attention in S^T layout over (qn, kc) blocks of [128 keys, 512 queries]:
    the head-pair shares one [128,1024] 2-bank PSUM tile (row groups 0-63 /
    64-127 run concurrently on the PE); one exp ACTIVATE per pair covers
    both heads; causality = skipping dead blocks + a 0/1 staircase mask
    multiply on diagonal blocks (VectorE).
  - ctx^T: the head-pair's two M=64 accumulations go to output partitions
    0-63 / 64-127 of one PSUM bank (column-group concurrency); softmax
    denominators ride as four concurrent M=1 matmuls (ones-column lhsT)
    into col positions {0,32,64,96} of a second bank.
  - 1/den = exp(-ln den) on ScalarE over the 4 head-rows at once; the
    broadcast across 64 partitions per head is ONE K=4 matmul against a
    block-ones "expand" matrix; normalization in-place on VectorE.
  - out partial = ctxt^T.T @ Wo, K=256 as two accumulating matmuls.

Scheduling: emission order interleaves the later Q/K rc-blocks and V chunks
into the ScalarE-bound attention loop as PE filler, and runs each qn's
normalize + output projection + DMA inside the next qn's attention, so the
PE never idles long enough for the HAM clock gate to re-throttle.

Infrastructure: the external walrus allows only ONE sync wait per
instruction; Tile emits more, so a post-pass hoists extra waits onto
single-wait no-ops and the TileContext closing drain is split into a chain
of single-wait drains.
"""

import numpy as np

import bass_rust
from bass_rust import ScopedClock
import concourse.bass as bass
import concourse.mybir as mybir
import concourse.tile as tile

F32 = mybir.dt.float32
BF16 = mybir.dt.bfloat16
F32R = mybir.dt.float32r
B, T, D = 2, 2048, 1024
NCORES = 8
P = 128          # partitions
FC = D // P      # 8 feature chunks
QW = 512         # query block width
QN = T // QW     # 4 query blocks
KC = T // P      # 16 key chunks
NH = 4           # heads per core
DK = 64
HW = NH * DK     # 256: per-core qkv width

# ---------------------------------------------------------------------------
# TileContext drain fix: the external walrus in this container allows only ONE
# sync wait per instruction, but Tile's closing drain packs one wait per active
# proc. Split it into a chain of single-wait drains (same semantics).
_PATCHED = False


def _patched_drain_and_barrier(self, tick_clock, wait_clock):
    nc = self.nc
    drain_inst = nc.sync.drain()
    wait_clock.add_sem_waits(
        drain_inst.ins, ScopedClock({None: tick_clock.global_clock})
    )
    si = drain_inst.ins.sync_info
    waits = list(si.on_wait) if si is not None else []
    if len(waits) > 1:
        si.on_wait = [waits[0]]
        drain_inst.ins.sync_info = si
        for w in waits[1:]:
            d2 = nc.sync.drain()
            si2 = d2.ins.sync_info
            if si2 is None:
                si2 = bass_rust.SyncInfo(on_wait=[w], on_update=[])
            else:
                si2.on_wait = [w]
            d2.ins.sync_info = si2
    nc.all_engine_barrier()
    assert self.sems is not None
    popped = nc._tile_sem_poison_stack.pop()
    assert popped is self._sem_poison
    nc.clear_and_free_semaphores(list(self.sems.allocated().values()))
    nc.all_engine_barrier()


def _apply_tile_patch():
    global _PATCHED
    if not _PATCHED:
        tile.TileContext._drain_and_barrier = _patched_drain_and_barrier
        _PATCHED = True


def _split_multi_waits(nc):
    """Post-pass: the external walrus accepts only 1 sync wait per
    instruction (2 for EventSemaphore). Tile emits more. Hoist extra waits
    onto same-engine no-ops inserted just before."""
    for f in nc.m.functions:
        for bb in f.blocks:
            new = []
            for ins in bb.instructions:
                si = ins.sync_info
                if si is not None:
                    cap = 2 if isinstance(ins, mybir.InstEventSemaphore) else 1
                    waits = list(si.on_wait)
                    if len(waits) > cap:
                        for w in waits[:-cap]:
                            nop = mybir.InstNoOp(
                                name=nc.get_next_instruction_name(),
                                engine=ins.engine,
                                sync_info=bass_rust.SyncInfo(
                                    on_wait=[w], on_update=[]
                                ),
                                bass_nofuse=True,
                            )
                            nc.register_instruction(nop, overwrite=True)
                            new.append(nop)
                        si.on_wait = waits[-cap:]
                        ins.sync_info = si
                new.append(ins)
            bb.instructions = new


# ---------------------------------------------------------------------------
_PROGRAM = None


def build_program():
    global _PROGRAM
    if _PROGRAM is not None:
        return _PROGRAM
    _apply_tile_patch()
    Exp = mybir.ActivationFunctionType.Exp
    Log = mybir.ActivationFunctionType.Ln

    nc = bass.Bass()
    xt_d = nc.declare_dram_parameter("xt", [D, T], BF16, isOutput=False)
    wq_d = nc.declare_dram_parameter("wq", [D, HW], BF16, isOutput=False)
    wk_d = nc.declare_dram_parameter("wk", [D, HW], BF16, isOutput=False)
    wv_d = nc.declare_dram_parameter("wv", [D, HW], BF16, isOutput=False)
    wo_d = nc.declare_dram_parameter("wo", [HW, D], BF16, isOutput=False)
    mask_d = nc.declare_dram_parameter("mask", [P, 4, 1024], BF16, isOutput=False)
    e4_d = nc.declare_dram_parameter("e4", [P, HW], F32R, isOutput=False)
    out_d = nc.declare_dram_parameter("out", [T, D], F32, isOutput=True)

    with tile.TileContext(nc) as tc:
        from contextlib import ExitStack

        ctx = ExitStack()
        with ctx:
            consts = ctx.enter_context(tc.tile_pool(name="consts", bufs=1))
            xt_pool = ctx.enter_context(tc.tile_pool(name="xt", bufs=8))
            qk_pool = ctx.enter_context(tc.tile_pool(name="qk", bufs=1))
            v_pool = ctx.enter_context(tc.tile_pool(name="v", bufs=1))
            exp_pool = ctx.enter_context(tc.tile_pool(name="exp", bufs=6))
            ctxt_pool = ctx.enter_context(tc.tile_pool(name="ctxt", bufs=1))
            den_pool = ctx.enter_context(tc.tile_pool(name="den", bufs=1))
            ob_pool = ctx.enter_context(tc.tile_pool(name="ob", bufs=3))

            ps_big = ctx.enter_context(
                tc.tile_pool(name="ps_big", bufs=2, space="PSUM")
            )
            ps_ctx = ctx.enter_context(
                tc.tile_pool(name="ps_ctx", bufs=2, space="PSUM")
            )
            ps_den = ctx.enter_context(
                tc.tile_pool(name="ps_den", bufs=2, space="PSUM")
            )

            # ---- constants ----
            wq_sb = consts.tile([P, FC, HW], BF16, tag="wq")
            wk_sb = consts.tile([P, FC, HW], BF16, tag="wk")
            wv_sb = consts.tile([P, FC, HW], BF16, tag="wv")
            wo_sb = consts.tile([P, 2, D], BF16, tag="wo")
            mask_sb = consts.tile([P, 4, 1024], BF16, tag="mask")
            e4_sb = consts.tile([P, HW], F32R, tag="e4")
            onescol = consts.tile([P, 1], BF16, tag="onescol")
            # xt chunks + wq/wk trigger first on the sync queue (they gate
            # the first projections); bulky later-phase consts go through the
            # gpsimd queue so their descriptor-gen does not delay xt.
            xts = []
            for fc in range(2):
                xt_t = xt_pool.tile([P, T], BF16, tag="xt", name=f"xt{fc}")
                nc.sync.dma_start(out=xt_t, in_=xt_d[fc * P : (fc + 1) * P, :])
                xts.append(xt_t)
            nc.sync.dma_start(out=wq_sb, in_=wq_d.rearrange("(f p) c -> p f c", p=P))
            nc.sync.dma_start(out=wk_sb, in_=wk_d.rearrange("(f p) c -> p f c", p=P))
            for fc in range(2, FC):
                xt_t = xt_pool.tile([P, T], BF16, tag="xt", name=f"xt{fc}")
                nc.sync.dma_start(out=xt_t, in_=xt_d[fc * P : (fc + 1) * P, :])
                xts.append(xt_t)
            nc.gpsimd.dma_start(out=wv_sb, in_=wv_d.rearrange("(f p) c -> p f c", p=P))
            nc.gpsimd.dma_start(out=wo_sb, in_=wo_d.rearrange("(k p) c -> p k c", p=P))
            nc.gpsimd.dma_start(out=mask_sb, in_=mask_d[:, :, :])
            nc.gpsimd.dma_start(out=e4_sb, in_=e4_d[:, :])
            nc.vector.memset(onescol, 1.0)

            # persistent SBUF tensors
            qt = [qk_pool.tile([P, T], BF16, tag=f"qt{h}", name=f"qt{h}") for h in range(2)]
            kt = [qk_pool.tile([P, T], BF16, tag=f"kt{h}", name=f"kt{h}") for h in range(2)]
            v_sb = v_pool.tile([P, KC, HW], BF16, tag="v")
            ctxt = [ctxt_pool.tile([P, T], BF16, tag=f"ctxt{k}", name=f"ctxt{k}") for k in range(2)]
            lnden = den_pool.tile([P, T], F32, tag="lnden")
            rcp = den_pool.tile([P, T], F32R, tag="rcp")

            def mm2(out, lhsT, rhs, start, stop, tp_row=0):
                """Col-split matmul: two concurrent M=64 halves on disjoint
                array column groups; each half's LDWEIGHTS hides under the
                other's stream."""
                for ci in range(2):
                    nc.tensor.matmul(
                        out[ci * DK : (ci + 1) * DK, :],
                        lhsT=lhsT[:, ci * DK : (ci + 1) * DK],
                        rhs=rhs,
                        start=start,
                        stop=stop,
                        skip_group_check=True,
                        tile_position=(tp_row, ci * DK),
                    )

            # ---- PE work units (emitted directly or as attention filler) ----
            def qk_unit(dst, w_sb, hp, rc):
                """One [128,512] Q^T or K^T block: 8 accumulating matmuls."""
                ps = ps_big.tile([P, 2 * QW], F32, tag="big", name=f"qk{id(dst)}{hp}{rc}")
                for fc in range(FC):
                    mm2(
                        ps[:, 0:QW],
                        w_sb[:, fc, hp * P : (hp + 1) * P],
                        xts[fc][:, rc * QW : (rc + 1) * QW],
                        start=(fc == 0),
                        stop=(fc == FC - 1),
                    )
                nc.vector.tensor_copy(dst[:, rc * QW : (rc + 1) * QW], ps[:, 0:QW])

            def v_unit(kc):
                """Two key chunks of V [128 tok, 256 dv each]: 16 matmuls."""
                ps = ps_big.tile([P, 2 * QW], F32, tag="big", name=f"v{kc}")
                for i in range(2):
                    for fc in range(FC):
                        mm2(
                            ps[:, i * HW : (i + 1) * HW],
                            xts[fc][:, (kc + i) * P : (kc + i + 1) * P],
                            wv_sb[:, fc, :],
                            start=(fc == 0),
                            stop=(fc == FC - 1),
                        )
                nc.vector.tensor_copy(
                    v_sb[:, kc : kc + 2, :],
                    ps[:, 0 : 2 * HW].rearrange("p (i c) -> p i c", i=2),
                )

            # ---- upfront projections: only the rc0 Q/K blocks ----
            for hp in range(2):
                qk_unit(qt[hp], wq_sb, hp, 0)
                qk_unit(kt[hp], wk_sb, hp, 0)

            # filler schedule: work units injected into qn's attention iters
            # (after each iteration's S matmuls, so they sit off the S->exp
            # critical path and fill the PE while ScalarE works)
            def qk_f(dst, w_sb, hp, rc):
                return lambda: qk_unit(dst, w_sb, hp, rc)

            fillers = {
                0: [lambda: v_unit(0), lambda: v_unit(2),
                    qk_f(qt[0], wq_sb, 0, 1), qk_f(kt[0], wk_sb, 0, 1),
                    qk_f(qt[1], wq_sb, 1, 1), qk_f(kt[1], wk_sb, 1, 1),
                    lambda: v_unit(4)],
                1: [qk_f(qt[0], wq_sb, 0, 2), qk_f(kt[0], wk_sb, 0, 2),
                    qk_f(qt[1], wq_sb, 1, 2), qk_f(kt[1], wk_sb, 1, 2),
                    lambda: v_unit(6), lambda: v_unit(8)],
                2: [qk_f(qt[0], wq_sb, 0, 3), qk_f(kt[0], wk_sb, 0, 3),
                    qk_f(qt[1], wq_sb, 1, 3), qk_f(kt[1], wk_sb, 1, 3),
                    lambda: v_unit(10), lambda: v_unit(12)],
                3: [lambda: v_unit(14)],
            }

            # deferred epilogue emitters, run interleaved into the next qn
            pending_epilogue = []

            def emit_epilogue(qn, ctx_ts, den_ps):
                steps = []
                qs = slice(qn * QW, (qn + 1) * QW)

                def s_evac():
                    for hp in range(2):
                        nc.vector.tensor_copy(ctxt[hp][:, qs], ctx_ts[hp])

                def s_rcp():
                    nc.scalar.activation(
                        out=lnden[:, qs], in_=den_ps, func=Log
                    )
                    nc.scalar.activation(
                        out=rcp[:, qs], in_=lnden[:, qs], func=Exp, scale=-1.0
                    )

                def s_norm():
                    bc = ps_big.tile([P, 2 * QW], F32, tag="big", name=f"bc{qn}")
                    for hp in range(2):
                        nc.tensor.matmul(
                            bc[:, hp * QW : (hp + 1) * QW],
                            lhsT=e4_sb[:, hp * P : (hp + 1) * P],
                            rhs=rcp[:, qs],
                            start=True,
                            stop=True,
                        )
                    for hp in range(2):
                        nc.vector.tensor_mul(
                            ctxt[hp][:, qs],
                            ctxt[hp][:, qs],
                            bc[:, hp * QW : (hp + 1) * QW],
                        )

                def s_oproj(rc):
                    def run():
                        ps = ps_big.tile(
                            [P, 2 * QW], F32, tag="big", name=f"op{qn}{rc}"
                        )
                        tok = qn * QW + rc * P
                        for kch in range(2):
                            for c2 in range(2):
                                mm2(
                                    ps[:, c2 * QW : (c2 + 1) * QW],
                                    ctxt[kch][:, tok : tok + P],
                                    wo_sb[:, kch, c2 * QW : (c2 + 1) * QW],
                                    start=(kch == 0),
                                    stop=(kch == 1),
                                )
                        ob = ob_pool.tile([P, D], F32, tag="ob")
                        nc.vector.tensor_copy(ob, ps)
                        nc.sync.dma_start(out=out_d[tok : tok + P, :], in_=ob)
                    return run

                steps.append(s_evac)
                steps.append(s_rcp)
                steps.append(s_norm)
                for rc in range(4):
                    steps.append(s_oproj(rc))
                return steps

            # ---- attention ----
            for qn in range(QN):
                nkc = 4 * (qn + 1)
                qs = slice(qn * QW, (qn + 1) * QW)
                ctx_ts = [
                    ps_ctx.tile([P, QW], F32, tag="ctx", name=f"ctx{qn}{hp}")
                    for hp in range(2)
                ]
                den_ps = ps_den.tile([P, QW], F32, tag="den", name=f"den{qn}")
                nc.vector.memset(den_ps, 1.0)
                fill = list(fillers[qn]) + pending_epilogue
                pending_epilogue = []
                # spread filler work across this qn's kc iterations
                nfill = len(fill)
                for kc in range(nkc):
                    j = kc - 4 * qn
                    es = []
                    for hp in range(2):
                        s_ps = ps_big.tile(
                            [P, 2 * QW], F32, tag="big", name=f"s{qn}{kc}{hp}"
                        )
                        for h2 in range(2):
                            mm2(
                                s_ps[:, h2 * QW : (h2 + 1) * QW],
                                kt[hp][h2 * DK : (h2 + 1) * DK, kc * P : (kc + 1) * P],
                                qt[hp][h2 * DK : (h2 + 1) * DK, qs],
                                start=True,
                                stop=True,
                                tp_row=h2 * DK,
                            )
                        e = exp_pool.tile([P, 2 * QW], BF16, tag="exp")
                        nc.scalar.activation(out=e, in_=s_ps, func=Exp, scale=0.125)
                        if j >= 0:  # diagonal: causal staircase mask
                            nc.vector.tensor_mul(e, e, mask_sb[:, j, :])
                        es.append(e)
                    lo = nfill * kc // nkc
                    hi = nfill * (kc + 1) // nkc
                    for f in fill[lo:hi]:
                        f()
                    first = kc == 0
                    last = kc == nkc - 1
                    for hp in range(2):
                        for h2 in range(2):
                            # start=True only for the bank's first write; the
                            # pair shares one bank (h2=1 overwrites-where-clear)
                            nc.tensor.matmul(
                                ctx_ts[hp][h2 * DK : (h2 + 1) * DK, :],
                                lhsT=v_sb[:, kc, (2 * hp + h2) * DK : (2 * hp + h2 + 1) * DK],
                                rhs=es[hp][:, h2 * QW : (h2 + 1) * QW],
                                start=first,
                                stop=last,
                                skip_group_check=True,
                            )
                    for hp in range(2):
                        for h2 in range(2):
                            h = 2 * hp + h2
                            nc.tensor.matmul(
                                den_ps[32 * h : 32 * h + 1, :],
                                lhsT=onescol,
                                rhs=es[hp][:, h2 * QW : (h2 + 1) * QW],
                                start=first,
                                stop=last,
                                skip_group_check=True,
                                tile_position=(0, 32 * h),
                            )
                pending_epilogue = emit_epilogue(qn, ctx_ts, den_ps)
            for f in pending_epilogue:
                f()

    _split_multi_waits(nc)
    _PROGRAM = nc
    return nc


def _make_mask():
    # mask4[i, j, 512h + q] = 1.0 if q >= 128j + i else 0.0  (shape [128, 4, 1024])
    i = np.arange(P)[:, None, None]
    j = np.arange(4)[None, :, None]
    q = np.arange(1024)[None, None, :] % 512
    return (q >= 128 * j + i).astype(np.float32)


def make_in_maps(x, Wq, Wk, Wv, Wo):
    import ml_dtypes

    nd = ml_dtypes.bfloat16
    x = np.asarray(x, dtype=np.float32)
    xt = np.ascontiguousarray(x.reshape(B * T, D).T).astype(nd)  # [1024, 4096]
    mask = _make_mask().astype(nd)
    pp = np.arange(P)[:, None]
    cc = np.arange(HW)[None, :]
    e4 = (pp == 32 * (cc // DK)).astype(np.float32)  # [128, 256] (f32r)
    Wq, Wk, Wv, Wo = (np.asarray(w, dtype=np.float32) for w in (Wq, Wk, Wv, Wo))
    in_maps = []
    for c in range(NCORES):
        b, hg = c // 4, c % 4
        cols = slice(hg * HW, (hg + 1) * HW)
        in_maps.append(
            {
                "xt": np.ascontiguousarray(xt[:, b * T : (b + 1) * T]),
                "wq": np.ascontiguousarray(Wq[:, cols]).astype(nd),
                "wk": np.ascontiguousarray(Wk[:, cols]).astype(nd),
                "wv": np.ascontiguousarray(Wv[:, cols]).astype(nd),
                "wo": np.ascontiguousarray(Wo[cols, :]).astype(nd),
                "mask": mask,
                "e4": e4,
            }
        )
    return in_maps


def kernel(x, Wq, Wk, Wv, Wo):
    from concourse.bass_utils import run_bass_kernel_spmd

    nc = build_program()
    in_maps = make_in_maps(x, Wq, Wk, Wv, Wo)
    res = run_bass_kernel_spmd(nc, in_maps, core_ids=list(range(NCORES)))
    out = np.zeros((B, T, D), dtype=np.float64)
    for c in range(NCORES):
        out[c // 4] += res.results[c]["out"]
    return out.astype(np.float32)


if __name__ == "__main__":
    rng = np.random.default_rng(0)
    s = 1.0 / np.sqrt(D)
    ins = {
        "x": rng.standard_normal((B, T, D)).astype(np.float32),
        "Wq": (rng.standard_normal((D, D)) * s).astype(np.float32),
        "Wk": (rng.standard_normal((D, D)) * s).astype(np.float32),
        "Wv": (rng.standard_normal((D, D)) * s).astype(np.float32),
        "Wo": (rng.standard_normal((D, D)) * (1.0 / np.sqrt(D))).astype(np.float32),
    }
    out = kernel(**ins)
    print("out", out.shape, out.dtype, float(np.abs(out).max()))
